# revision 40
# baseline (speedup 1.0000x reference)
"""Trainium2 Bass kernel for the scatter_memory GRU memory-update module.

Computation (torch GRUCell semantics, chunk order r, z, n):
    current = memory[node_ids]                       # [B, H] gather
    gi = messages @ W_ih.T + b_ih ; gh = current @ W_hh.T + b_hh
    r = sigmoid(gi_r + gh_r) ; z = sigmoid(gi_z + gh_z)
    n = tanh(gi_n + r * gh_n)
    updated = (1 - z) * n + z * current
    new_memory = memory.at[node_ids].set(updated)    # scatter
"""

import os
import sys

import numpy as np

for _p in ("/opt/trn_rl_repo", "/root/.axon_site/_ro/trn_rl_repo"):
    if os.path.isdir(_p) and _p not in sys.path:
        sys.path.insert(0, _p)

# bass_utils imports antenv.axon_hooks unconditionally when BASS_TRACE is
# set; provide a stub registry if the agent image's antenv lacks it (the
# NTFF hook then stays None and tracing is skipped instead of crashing).
try:
    import antenv.axon_hooks  # noqa: F401
except Exception:
    import types as _types

    _m = _types.ModuleType("antenv.axon_hooks")
    _m._hook = None
    _m.set_axon_ntff_profile_hook = lambda h: setattr(_m, "_hook", h)
    _m.get_axon_ntff_profile_hook = lambda: _m._hook
    sys.modules["antenv.axon_hooks"] = _m

import ml_dtypes
from contextlib import ExitStack

import concourse.bass as bass
import concourse.tile as tile
from concourse import mybir
from concourse.bass_utils import run_bass_kernel_spmd

BF16 = ml_dtypes.bfloat16
F8 = ml_dtypes.float8_e4m3          # TRN fp8e4: e4m3 with +-240 max
import json as _json

N_CORES = 8
H = 128
NTILE = 1024         # batch columns per PSUM tile (2 banks of fp32 per gate)
DMA_CHUNK = 2048     # batch columns per input DMA

# exposed for test harnesses
LAST_RESULT = None

_NC_CACHE = {}


def _dma_chunks(bpc: int) -> list[tuple[int, int]]:
    """Input DMA schedule: two 1024 ramp chunks (compute starts after the
    first), then wide transfers.  Tile count is minimized — every tile
    costs ~3 fixed-overhead ACT ops on the pacing engine."""
    sizes = []
    pos = 0
    for ramp in (512, 1024):
        if pos + ramp <= bpc:
            sizes.append(ramp)
            pos += ramp
    while pos < bpc:
        s = min(DMA_CHUNK, bpc - pos)
        sizes.append(s)
        pos += s
    out = []
    pos = 0
    for s in sizes:
        out.append((pos, s))
        pos += s
    assert pos == bpc
    return out


def _tiles(bpc: int) -> list[tuple[int, int]]:
    """Compute-tile schedule: 1024-wide steady state (PSUM capacity),
    tapered tail so the final serial chain is short.  Tiles never cross
    an input-DMA chunk boundary."""
    out = []
    for c0, csz in _dma_chunks(bpc):
        for p in range(c0, c0 + csz, NTILE):
            out.append((p, min(NTILE, c0 + csz - p)))
    assert sum(s for _, s in out) == bpc
    return out


def _elide_redundant_waits(bir: dict) -> dict:
    """Transitive reduction of semaphore waits (vector clocks).

    Tile's dependency semaphores are monotonic counters (sem-inc /
    sem-ge-imm).  A wait (S >= V) is redundant when the waiting engine
    already knows S >= V — either from an earlier wait on its own
    stream, or transitively: if it waited on engine E's counter at a
    point where E itself had already waited for S >= V.  Each elided
    wait saves ~90ns of engine issue time; the savings land on the
    pacing engines.  DMA-queue sems (increments not visible as
    on_update) and non-monotonic sems (sem-dec barriers) are never used
    as transitive carriers / never elided.
    """
    import bisect

    bad = set()
    for fn in bir.get("functions", []):
        for blk in fn.get("blocks", []):
            for inst in blk.get("instructions", []):
                si = inst.get("sync_info") or {}
                for u in si.get("on_update") or []:
                    if u.get("update_mode") != "sem-inc":
                        bad.add(u["id"])

    for fn in bir.get("functions", []):
        for blk in fn.get("blocks", []):
            clock: dict = {}     # engine -> {sem_id: guaranteed value}
            counters: dict = {}  # sem_id -> running count
            snaps: dict = {}     # sem_id -> ([counts], [clock dicts])
            for inst in blk.get("instructions", []):
                e = inst["engine"]
                si = inst.get("sync_info") or {}
                know = clock.setdefault(e, {})
                ow = si.get("on_wait") or []
                kept = []
                for w in ow:
                    sid = w["id"]
                    mono = w.get("wait_mode") == "sem-ge-imm" and sid not in bad
                    if mono and know.get(sid, -1) >= w["wait_value"]:
                        continue
                    kept.append(w)
                    if mono:
                        # inherit the incrementer's knowledge at that count
                        sn = snaps.get(sid)
                        if sn is not None:
                            i = bisect.bisect_left(sn[0], w["wait_value"])
                            if i < len(sn[0]):
                                for s2, v2 in sn[1][i].items():
                                    if know.get(s2, -1) < v2:
                                        know[s2] = v2
                        if know.get(sid, -1) < w["wait_value"]:
                            know[sid] = w["wait_value"]
                if si:
                    si["on_wait"] = kept
                for u in si.get("on_update") or []:
                    sid = u["id"]
                    if u.get("update_mode") == "sem-inc" and sid not in bad:
                        c = counters.get(sid, 0) + u.get("update_value", 1)
                        counters[sid] = c
                        if know.get(sid, -1) < c:
                            know[sid] = c
                        sn = snaps.setdefault(sid, ([], []))
                        sn[0].append(c)
                        sn[1].append(dict(know))
    return bir


def _elide_duplicate_ldweights(bir: dict) -> dict:
    """Drop PE Ldweights whose stationary AP is identical to the
    previous Ldweights with no intervening weight change (weights are
    loaded once into SBUF and never rewritten).  The PE array keeps its
    stationary registers across matmuls, so the reload is a no-op that
    still costs ~100ns of PE issue time.  Waits on an elided Ldweights
    are migrated to the next PE instruction."""
    for fn in bir.get("functions", []):
        for blk in fn.get("blocks", []):
            out = []
            last_w = None
            pend = []
            for inst in blk.get("instructions", []):
                if inst["engine"] != "PE":
                    out.append(inst)
                    continue
                if inst["opcode"] == "Ldweights":
                    key = _json.dumps(inst.get("ins"), sort_keys=True)
                    si = inst.get("sync_info") or {}
                    if key == last_w:
                        pend.extend(si.get("on_wait") or [])
                        continue
                    last_w = key
                    out.append(inst)
                elif inst["opcode"] == "Matmult":
                    if pend:
                        si = inst.setdefault("sync_info",
                                             {"on_update": [], "on_wait": []})
                        si["on_wait"] = list(si.get("on_wait") or []) + pend
                        pend = []
                    out.append(inst)
                else:
                    out.append(inst)
            assert not pend
            blk["instructions"] = out
    return bir


def _split_sync_waits(bir: dict) -> dict:
    """Hoist extra per-instruction semaphore waits into standalone
    EventSemaphore instructions.

    The walrus build in this container encodes at most ONE sync wait per
    instruction ("Too many sync wait commands" otherwise); Tile attaches
    one wait per dependency.  An engine-level standalone wait immediately
    before the instruction is semantically identical (the engine stalls
    either way), so keep the last wait inline and hoist the rest.
    """
    n = 0
    for fn in bir.get("functions", []):
        for blk in fn.get("blocks", []):
            out = []
            for inst in blk.get("instructions", []):
                si = inst.get("sync_info") or {}
                ow = si.get("on_wait") or []
                if len(ow) > 1:
                    for w in ow[:-1]:
                        n += 1
                        out.append({
                            "debug": inst.get("debug", 0),
                            "engine": inst["engine"],
                            "ins": [],
                            "outs": [],
                            "name": f"hoistw_{n}_{inst['name']}",
                            "opcode": "EventSemaphore",
                            "sync_info": {"on_update": [], "on_wait": [w]},
                        })
                    si["on_wait"] = [ow[-1]]
                out.append(inst)
            blk["instructions"] = out
    return bir


def _patch_json(nc: bass.Bass) -> None:
    orig = nc.to_json_bytes

    def patched() -> bytes:
        return _json.dumps(
            _split_sync_waits(
                _elide_duplicate_ldweights(
                    _elide_redundant_waits(_json.loads(orig()))))
        ).encode()

    nc.to_json_bytes = patched


def _build_nc(bpc: int) -> bass.Bass:
    """Bass program for one core: GRU over a [H, bpc] feature-major shard.

    Per-input-chunk byte layout (uint8 dram tensor `inp`, 4*csz bytes):
        [x_f8 (csz) | h_f8 (csz) | h_bf16 (2*csz)]
    x_f8|h_f8 doubles as the DoubleRow rhs [K, 2, N] (k-tile stride csz).

    Gate matmuls: r/z via fp8 DoubleRow (merges the W_ih@x and W_hh@h
    contractions into one pass at 2 elem/cycle); i_n via plain fp8;
    h_n in bf16 (its path feeds tanh directly, keep it accurate).

    The DVE STT writes t = (h_n + b_hn)*r back INTO the p_hn PSUM bank
    (f32, in-place); the i_n matmuls then accumulate onto it with
    start=False one period later, so `pre = t + i_n` happens on the PE
    for free and tanh reads the finished pre-activation from PSUM.

    Software-pipelined per-engine stream order (period k):
        PE : inMM(k-1), rMM(k), zMM(k), hnMM(k)
        ACT: tanh(k-1), sig_r(k), sig_z(k)
        DVE: m(k-2), out(k-2), d(k-1), STT(k)
    so every instruction's dependencies were issued >= 1 period ago.
    """
    assert bpc % 256 == 0
    f32 = mybir.dt.float32
    bf16 = mybir.dt.bfloat16
    f8e4 = mybir.dt.float8e4
    u8 = mybir.dt.uint8
    sig = mybir.ActivationFunctionType.Sigmoid
    tanh = mybir.ActivationFunctionType.Tanh
    add_op = mybir.AluOpType.add
    mult_op = mybir.AluOpType.mult
    DR = mybir.MatmulPerfMode.DoubleRow

    nc = bass.Bass()
    inp = nc.declare_dram_parameter("inp", [H, 4 * bpc], u8, isOutput=False)
    # packed [w_ih_r | w_hh_r | biases | w_ih_z | w_hh_z | w_ih_n | w_hh_n]
    # (bf16 bytes) — everything the FIRST matmul + sigmoid need is a
    # contiguous 66KB prefix.
    # bias columns: 0 = b_ih_r + b_hh_r, 1 = b_ih_z + b_hh_z, 2 = b_hh_n, 3 = b_ih_n
    wb = nc.declare_dram_parameter("wb", [H, 12 * H + 8], u8, isOutput=False)
    outT = nc.declare_dram_parameter("outT", [H, bpc], bf16, isOutput=True)

    with ExitStack() as ctx:
        tc = ctx.enter_context(tile.TileContext(nc))
        singles = ctx.enter_context(tc.tile_pool(name="singles", bufs=1))
        io = ctx.enter_context(tc.tile_pool(name="io", bufs=1))
        mids = ctx.enter_context(tc.tile_pool(name="mids", bufs=6))
        outs = ctx.enter_context(tc.tile_pool(name="outs", bufs=4))
        psum = ctx.enter_context(tc.tile_pool(name="psum", bufs=1, space="PSUM"))

        # weights + biases land first via sync-HWDGE; the first descriptor
        # carries only what the first matmul + sigmoid need (r-gate
        # weights + biases, 66KB) so compute starts ~1us earlier, the
        # second carries the z/n weights (needed one period later)
        wb_sb = singles.tile([H, 12 * H + 8], u8)
        cut = 4 * H + 8              # bytes: w_ih_r + w_hh_r + biases
        nc.sync.dma_start(out=wb_sb[:, :cut], in_=wb[:, :cut])
        nc.sync.dma_start(out=wb_sb[:, cut:], in_=wb[:, cut:])
        wb_bf = wb_sb[:, :].bitcast(bf16)                            # [H, 6H+4]
        w_ihr = wb_bf[:, 0:H]
        w_hhr = wb_bf[:, H : 2 * H]
        b_sb = wb_bf[:, 2 * H : 2 * H + 4]
        w_ihz = wb_bf[:, 2 * H + 4 : 3 * H + 4]
        w_hhz = wb_bf[:, 3 * H + 4 : 4 * H + 4]
        w_ihn = wb_bf[:, 4 * H + 4 : 5 * H + 4]
        w_hhn = wb_bf[:, 5 * H + 4 : 6 * H + 4]

        # dummy sigmoid fires the ~2.7us ACT table load immediately, so it
        # overlaps the DMA ramp instead of stalling the first real sigmoid
        # (memset first so it doesn't wait on any DMA)
        warm_sb = singles.tile([H, 1], f32)
        nc.vector.memset(warm_sb, 0.0)
        nc.scalar.activation(out=warm_sb, in_=warm_sb,
                             func=sig, bias=0.0, scale=1.0)

        # Pre-issue EVERY input DMA before any compute/output instruction
        # lands in the sync queue: descriptor generation costs ~0.7us on
        # the issuing engine, and an out-DMA interleaved in the stream
        # would stall all later in-DMA issues behind that chunk's compute.
        # All chunks get distinct tiles (whole input fits in SBUF).
        parts = []
        for ci, (c0, csz) in enumerate(_dma_chunks(bpc)):
            t = io.tile([H, 4 * csz], u8, tag=f"c{ci}")
            nc.sync.dma_start(out=t, in_=inp[:, 4 * c0 : 4 * c0 + 4 * csz])
            parts.append((c0, csz, t))

        def views(lo, n):
            """(x_bf [H,n], h_bf [H,n]) for cols [lo, lo+n)."""
            for c0, csz, t in parts:
                if c0 <= lo and lo + n <= c0 + csz:
                    r0 = lo - c0
                    return (t[:, 0 : 2 * csz].bitcast(bf16)[:, r0 : r0 + n],
                            t[:, 2 * csz : 4 * csz].bitcast(bf16)[:, r0 : r0 + n])
            raise AssertionError((lo, n))

        tiles = _tiles(bpc)
        n_tiles = len(tiles)
        state = {}

        # pair consecutive full tiles from the same DMA chunk: the z/n
        # buffers and the whole blend (d, m, out, output DMA) then run at
        # [H, 2048] granularity — one instruction instead of two saves
        # the per-op fixed cost + semaphores on the pacing engines.
        def chunk_of(pos):
            for c0, csz, _ in parts:
                if c0 <= pos < c0 + csz:
                    return c0
            raise AssertionError(pos)

        pair_even, pair_odd = set(), set()
        i = 0
        while i + 1 < n_tiles:
            if (tiles[i][1] == NTILE and tiles[i + 1][1] == NTILE
                    and chunk_of(tiles[i][0]) == chunk_of(tiles[i + 1][0])):
                pair_even.add(i)
                pair_odd.add(i + 1)
                i += 2
            else:
                i += 1
        # tanh pairing: consecutive full tiles share one [H, 2048] p_hn
        # pair tile (4 banks, same budget as bufs=2) and ONE tanh over
        # both halves — same bias b_ihn, so the sigma bias blocker does
        # not apply.  Saves ~450ns of ACT fixed cost per pair.
        hn_even, hn_odd = set(), set()
        i = 0
        while i + 1 < n_tiles:
            if tiles[i][1] == NTILE and tiles[i + 1][1] == NTILE:
                hn_even.add(i)
                hn_odd.add(i + 1)
                i += 2
            else:
                i += 1
        hn_ctx = {}

        # measured: pair-wide blend (2048-wide d/m/out) bursts the DVE
        # and costs ~1.5us; keep only the output-DMA pairing.  The last
        # pair stays unpaired so the final transfers are small.
        dma_pair_even, dma_pair_odd = pair_even, pair_odd
        if dma_pair_even:
            le = max(dma_pair_even)
            dma_pair_even = dma_pair_even - {le}
            dma_pair_odd = dma_pair_odd - {le + 1}
        pair_even, pair_odd = set(), set()
        o_ctx = {}

        def stage_front(ti):
            t0, tsz = tiles[ti]
            x_sb, h_sb = views(t0, tsz)

            p_r = psum.tile([H, tsz], f32, tag="p_r")
            p_z = psum.tile([H, tsz], f32, tag="p_z")
            if ti in hn_even:
                p_hnp = psum.tile([H, 2 * NTILE], f32, tag="p_hn")
                hn_ctx[ti + 1] = p_hnp
                p_hn = p_hnp[:, 0:NTILE]
            elif ti in hn_odd:
                p_hnp = hn_ctx.pop(ti)
                p_hn = p_hnp[:, NTILE : 2 * NTILE]
            else:
                p_hnp = psum.tile([H, 2 * NTILE], f32, tag="p_hn")
                p_hn = p_hnp[:, 0:tsz]

            # weight-major order: back-to-back matmuls share stationary
            # weights, so the BIR pass below drops the duplicate
            # LDWEIGHTS (~100ns of PE issue each)
            for wih_g, whh_g, dst in ((w_ihr, w_hhr, p_r),
                                      (w_ihz, w_hhz, p_z)):
                for src_sb, w_sb, st in (
                    (x_sb, wih_g, True),
                    (h_sb, whh_g, False),
                ):
                    for q0 in range(0, tsz, 512):
                        qs = slice(q0, q0 + min(512, tsz - q0))
                        nc.tensor.matmul(dst[:, qs], w_sb, src_sb[:, qs],
                                         start=st, stop=not st,
                                         skip_group_check=True)
            for q0 in range(0, tsz, 512):
                qs = slice(q0, q0 + min(512, tsz - q0))
                nc.tensor.matmul(p_hn[:, qs], w_hhn,
                                 h_sb[:, qs], start=True, stop=False,
                                 skip_group_check=True)

            r_t = mids.tile([H, tsz], bf16, tag="r")
            if ti in pair_even:
                z_pair = mids.tile([H, 2 * NTILE], bf16, tag="z")
                state[ti] = st = {"z_pair": z_pair}
                z_t = z_pair[:, 0:NTILE]
            elif ti in pair_odd:
                st = state[ti] = {}
                z_t = state[ti - 1]["z_pair"][:, NTILE : 2 * NTILE]
            else:
                st = state[ti] = {}
                z_t = mids.tile([H, tsz], bf16, tag="z")
            nc.scalar.activation(out=r_t, in_=p_r, func=sig,
                                 bias=b_sb[:, 0:1], scale=1.0)
            nc.scalar.activation(out=z_t, in_=p_z, func=sig,
                                 bias=b_sb[:, 1:2], scale=1.0)

            # in-place: p_hn <- (p_hn + b_hn) * r   (f32, stays in PSUM)
            nc.vector.scalar_tensor_tensor(
                out=p_hn, in0=p_hn, scalar=b_sb[:, 2:3], in1=r_t,
                op0=add_op, op1=mult_op)
            st.update({"h": h_sb, "x": x_sb, "z": z_t, "p_hn": p_hn,
                       "p_hnp": p_hnp})

        def stage_in_mm(ti):
            """i_n matmuls accumulate onto the STT result (one period
            after front; first in the PE stream so tanh unblocks early)."""
            t0, tsz = tiles[ti]
            st = state[ti]
            p_hn, x_sb = st["p_hn"], st["x"]
            for q0 in range(0, tsz, 512):
                qs = slice(q0, q0 + min(512, tsz - q0))
                nc.tensor.matmul(p_hn[:, qs], w_ihn,
                                 x_sb[:, qs], start=False, stop=True,
                                 skip_group_check=True)

        def stage_tanh(ti):
            t0, tsz = tiles[ti]
            st = state[ti]
            if ti in hn_even:
                return               # one tanh per pair, at the odd member
            if ti in hn_odd:
                n_pair = mids.tile([H, 2 * NTILE], bf16, tag="n")
                nc.scalar.activation(out=n_pair,
                                     in_=st["p_hnp"][:, 0 : 2 * NTILE],
                                     func=tanh, bias=b_sb[:, 3:4], scale=1.0)
                state[ti - 1]["n"] = n_pair[:, 0:NTILE]
                st["n"] = n_pair[:, NTILE : 2 * NTILE]
                return
            n_t = mids.tile([H, tsz], bf16, tag="n")
            nc.scalar.activation(out=n_t, in_=st["p_hn"], func=tanh,
                                 bias=b_sb[:, 3:4], scale=1.0)
            st["n"] = n_t

        def stage_d(ti):
            if ti in hn_even:
                return               # n not ready until the pair tanh
            todo = (ti - 1, ti) if ti in hn_odd else (ti,)
            for tj in todo:
                t0, tsz = tiles[tj]
                st = state[tj]
                d_t = mids.tile([H, tsz], bf16, tag="d")
                nc.vector.tensor_sub(out=d_t, in0=st["h"], in1=st["n"])
                st["d"] = d_t

        def stage_blend(ti):
            """d = h-n, m = z*d, out = n + m, DMA (two periods after
            front).  For paired tiles everything runs once per pair at
            [H, 2048]."""
            t0, tsz = tiles[ti]
            if ti in pair_even:
                return               # odd member does the work
            if ti in pair_odd:
                ste = state.pop(ti - 1)
                state.pop(ti)
                z_pair = ste["z_pair"]
                n_pair = ste["n_pair"]
                e0 = tiles[ti - 1][0]
                h_pair = views(e0, 2 * NTILE)[1]
                d_t = mids.tile([H, 2 * NTILE], bf16, tag="d")
                m_t = mids.tile([H, 2 * NTILE], bf16, tag="m")
                o_t = outs.tile([H, 2 * NTILE], bf16, tag="o")
                nc.vector.tensor_sub(out=d_t, in0=h_pair, in1=n_pair)
                nc.vector.tensor_mul(out=m_t, in0=z_pair, in1=d_t)
                nc.vector.tensor_add(out=o_t, in0=n_pair, in1=m_t)
                nc.sync.dma_start(out=outT[:, e0 : e0 + 2 * NTILE], in_=o_t)
                return
            st = state.pop(ti)
            m_t = mids.tile([H, tsz], bf16, tag="m")
            nc.vector.tensor_mul(out=m_t, in0=st["z"], in1=st["d"])
            if ti in dma_pair_even:
                o_pair = outs.tile([H, 2 * NTILE], bf16, tag="o")
                o_ctx[ti + 1] = o_pair
                nc.vector.tensor_add(out=o_pair[:, 0:NTILE], in0=st["n"],
                                     in1=m_t)
            elif ti in dma_pair_odd:
                o_pair = o_ctx.pop(ti)
                nc.vector.tensor_add(out=o_pair[:, NTILE : 2 * NTILE],
                                     in0=st["n"], in1=m_t)
                nc.sync.dma_start(
                    out=outT[:, t0 - NTILE : t0 + tsz], in_=o_pair)
            else:
                o_t = outs.tile([H, tsz], bf16, tag="o")
                nc.vector.tensor_add(out=o_t, in0=st["n"], in1=m_t)
                nc.sync.dma_start(out=outT[:, t0 : t0 + tsz], in_=o_t)

        for ti in range(n_tiles):
            if ti >= 1:
                stage_in_mm(ti - 1)
                stage_tanh(ti - 1)
                stage_d(ti - 1)
            if ti >= 2:
                stage_blend(ti - 2)
            stage_front(ti)
        stage_in_mm(n_tiles - 1)
        stage_tanh(n_tiles - 1)
        stage_d(n_tiles - 1)
        stage_blend(n_tiles - 2)
        stage_blend(n_tiles - 1)

    _patch_json(nc)
    return nc


def _get_nc(bpc: int) -> bass.Bass:
    if bpc not in _NC_CACHE:
        _NC_CACHE[bpc] = _build_nc(bpc)
    return _NC_CACHE[bpc]


def kernel(node_ids, messages, memory, W_ih, W_hh, b_ih, b_hh):
    global LAST_RESULT
    node_ids = np.asarray(node_ids)
    messages = np.asarray(messages, dtype=np.float32)
    memory = np.asarray(memory, dtype=np.float32)
    W_ih = np.asarray(W_ih, dtype=np.float32)
    W_hh = np.asarray(W_hh, dtype=np.float32)
    b_ih = np.asarray(b_ih, dtype=np.float32)
    b_hh = np.asarray(b_hh, dtype=np.float32)

    B = node_ids.shape[0]
    per = -(-B // N_CORES)                       # rows per core (unpadded)
    bpc = -(-per // 256) * 256                   # padded to 256 multiple
    nc = _get_nc(bpc)
    chunks = _dma_chunks(bpc)

    current = memory[node_ids]                   # [B, H] host gather

    # weights: [w_ih_r | w_hh_r | biases | w_ih_z | w_hh_z | w_ih_n | w_hh_n]
    w_ihT = W_ih.T
    w_hhT = W_hh.T
    wbf = np.empty((H, 6 * H + 4), dtype=np.float32)
    wbf[:, 0:H] = w_ihT[:, 0:H]
    wbf[:, H : 2 * H] = w_hhT[:, 0:H]
    wbf[:, 2 * H + 0] = b_ih[0:H] + b_hh[0:H]
    wbf[:, 2 * H + 1] = b_ih[H : 2 * H] + b_hh[H : 2 * H]
    wbf[:, 2 * H + 2] = b_hh[2 * H : 3 * H]
    wbf[:, 2 * H + 3] = b_ih[2 * H : 3 * H]
    wbf[:, 2 * H + 4 : 3 * H + 4] = w_ihT[:, H : 2 * H]
    wbf[:, 3 * H + 4 : 4 * H + 4] = w_hhT[:, H : 2 * H]
    wbf[:, 4 * H + 4 : 5 * H + 4] = w_ihT[:, 2 * H : 3 * H]
    wbf[:, 5 * H + 4 : 6 * H + 4] = w_hhT[:, 2 * H : 3 * H]
    wb = wbf.astype(BF16).view(np.uint8)

    in_maps = []
    for c in range(N_CORES):
        lo = c * per
        hi = min(lo + per, B)
        xT = np.zeros((H, bpc), dtype=np.float32)
        hT = np.zeros((H, bpc), dtype=np.float32)
        if hi > lo:
            xT[:, : hi - lo] = messages[lo:hi].T
            hT[:, : hi - lo] = current[lo:hi].T
        x_bf = xT.astype(BF16)
        h_bf = hT.astype(BF16)
        inp = np.empty((H, 4 * bpc), dtype=np.uint8)
        for c0, csz in chunks:
            o = 4 * c0
            inp[:, o : o + 2 * csz] = np.ascontiguousarray(
                x_bf[:, c0 : c0 + csz]).view(np.uint8)
            inp[:, o + 2 * csz : o + 4 * csz] = np.ascontiguousarray(
                h_bf[:, c0 : c0 + csz]).view(np.uint8)
        in_maps.append({"inp": inp, "wb": wb})

    res = run_bass_kernel_spmd(nc, in_maps, list(range(N_CORES)))
    LAST_RESULT = res

    updated = np.empty((B, H), dtype=np.float32)
    for c in range(N_CORES):
        lo = c * per
        hi = min(lo + per, B)
        if hi > lo:
            updated[lo:hi] = res.results[c]["outT"][:, : hi - lo].T.astype(np.float32)

    new_memory = memory.copy()
    new_memory[node_ids] = updated
    return new_memory


# revision 41
# speedup vs baseline: 1.4944x; 1.4944x over previous
"""Trainium2 Bass kernel for the scatter_memory GRU memory-update module.

Computation (torch GRUCell semantics, chunk order r, z, n):
    current = memory[node_ids]                       # [B, H] gather
    gi = messages @ W_ih.T + b_ih ; gh = current @ W_hh.T + b_hh
    r = sigmoid(gi_r + gh_r) ; z = sigmoid(gi_z + gh_z)
    n = tanh(gi_n + r * gh_n)
    updated = (1 - z) * n + z * current
    new_memory = memory.at[node_ids].set(updated)    # scatter
"""

import os
import sys

import numpy as np

for _p in ("/opt/trn_rl_repo", "/root/.axon_site/_ro/trn_rl_repo"):
    if os.path.isdir(_p) and _p not in sys.path:
        sys.path.insert(0, _p)

# bass_utils imports antenv.axon_hooks unconditionally when BASS_TRACE is
# set; provide a stub registry if the agent image's antenv lacks it (the
# NTFF hook then stays None and tracing is skipped instead of crashing).
try:
    import antenv.axon_hooks  # noqa: F401
except Exception:
    import types as _types

    _m = _types.ModuleType("antenv.axon_hooks")
    _m._hook = None
    _m.set_axon_ntff_profile_hook = lambda h: setattr(_m, "_hook", h)
    _m.get_axon_ntff_profile_hook = lambda: _m._hook
    sys.modules["antenv.axon_hooks"] = _m

import ml_dtypes
from contextlib import ExitStack

import concourse.bass as bass
import concourse.tile as tile
from concourse import mybir
from concourse.bass_utils import run_bass_kernel_spmd

BF16 = ml_dtypes.bfloat16
F8 = ml_dtypes.float8_e4m3          # TRN fp8e4: e4m3 with +-240 max
import json as _json

N_CORES = 8
H = 128
NTILE = 1024         # batch columns per PSUM tile (2 banks of fp32 per gate)
DMA_CHUNK = 2048     # batch columns per input DMA

# exposed for test harnesses
LAST_RESULT = None

_NC_CACHE = {}


def _dma_chunks(bpc: int) -> list[tuple[int, int]]:
    """Input DMA schedule: two 1024 ramp chunks (compute starts after the
    first), then wide transfers.  Tile count is minimized — every tile
    costs ~3 fixed-overhead ACT ops on the pacing engine."""
    sizes = []
    pos = 0
    for ramp in (512, 1024):
        if pos + ramp <= bpc:
            sizes.append(ramp)
            pos += ramp
    while pos < bpc:
        s = min(DMA_CHUNK, bpc - pos)
        sizes.append(s)
        pos += s
    out = []
    pos = 0
    for s in sizes:
        out.append((pos, s))
        pos += s
    assert pos == bpc
    return out


def _tiles(bpc: int) -> list[tuple[int, int]]:
    """Compute-tile schedule: 1024-wide steady state (PSUM capacity),
    tapered tail so the final serial chain is short.  Tiles never cross
    an input-DMA chunk boundary."""
    out = []
    for c0, csz in _dma_chunks(bpc):
        for p in range(c0, c0 + csz, NTILE):
            out.append((p, min(NTILE, c0 + csz - p)))
    assert sum(s for _, s in out) == bpc
    return out


def _elide_redundant_waits(bir: dict) -> dict:
    """Transitive reduction of semaphore waits (vector clocks).

    Tile's dependency semaphores are monotonic counters (sem-inc /
    sem-ge-imm).  A wait (S >= V) is redundant when the waiting engine
    already knows S >= V — either from an earlier wait on its own
    stream, or transitively: if it waited on engine E's counter at a
    point where E itself had already waited for S >= V.  Each elided
    wait saves ~90ns of engine issue time; the savings land on the
    pacing engines.  DMA-queue sems (increments not visible as
    on_update) and non-monotonic sems (sem-dec barriers) are never used
    as transitive carriers / never elided.
    """
    import bisect

    bad = set()
    for fn in bir.get("functions", []):
        for blk in fn.get("blocks", []):
            for inst in blk.get("instructions", []):
                si = inst.get("sync_info") or {}
                for u in si.get("on_update") or []:
                    if u.get("update_mode") != "sem-inc":
                        bad.add(u["id"])

    for fn in bir.get("functions", []):
        for blk in fn.get("blocks", []):
            clock: dict = {}     # engine -> {sem_id: guaranteed value}
            counters: dict = {}  # sem_id -> running count
            snaps: dict = {}     # sem_id -> ([counts], [clock dicts])
            for inst in blk.get("instructions", []):
                e = inst["engine"]
                si = inst.get("sync_info") or {}
                know = clock.setdefault(e, {})
                ow = si.get("on_wait") or []
                kept = []
                for w in ow:
                    sid = w["id"]
                    mono = w.get("wait_mode") == "sem-ge-imm" and sid not in bad
                    if mono and know.get(sid, -1) >= w["wait_value"]:
                        continue
                    kept.append(w)
                    if mono:
                        # inherit the incrementer's knowledge at that count
                        sn = snaps.get(sid)
                        if sn is not None:
                            i = bisect.bisect_left(sn[0], w["wait_value"])
                            if i < len(sn[0]):
                                for s2, v2 in sn[1][i].items():
                                    if know.get(s2, -1) < v2:
                                        know[s2] = v2
                        if know.get(sid, -1) < w["wait_value"]:
                            know[sid] = w["wait_value"]
                if si:
                    si["on_wait"] = kept
                for u in si.get("on_update") or []:
                    sid = u["id"]
                    if u.get("update_mode") == "sem-inc" and sid not in bad:
                        c = counters.get(sid, 0) + u.get("update_value", 1)
                        counters[sid] = c
                        if know.get(sid, -1) < c:
                            know[sid] = c
                        sn = snaps.setdefault(sid, ([], []))
                        sn[0].append(c)
                        sn[1].append(dict(know))
    return bir


def _elide_duplicate_ldweights(bir: dict) -> dict:
    """Drop PE Ldweights whose stationary AP is identical to the
    previous Ldweights with no intervening weight change (weights are
    loaded once into SBUF and never rewritten).  The PE array keeps its
    stationary registers across matmuls, so the reload is a no-op that
    still costs ~100ns of PE issue time.  Waits on an elided Ldweights
    are migrated to the next PE instruction."""
    for fn in bir.get("functions", []):
        for blk in fn.get("blocks", []):
            out = []
            last_w = None
            pend = []
            for inst in blk.get("instructions", []):
                if inst["engine"] != "PE":
                    out.append(inst)
                    continue
                if inst["opcode"] == "Ldweights":
                    key = _json.dumps(inst.get("ins"), sort_keys=True)
                    si = inst.get("sync_info") or {}
                    if key == last_w:
                        pend.extend(si.get("on_wait") or [])
                        continue
                    last_w = key
                    out.append(inst)
                elif inst["opcode"] == "Matmult":
                    if pend:
                        si = inst.setdefault("sync_info",
                                             {"on_update": [], "on_wait": []})
                        si["on_wait"] = list(si.get("on_wait") or []) + pend
                        pend = []
                    out.append(inst)
                else:
                    out.append(inst)
            assert not pend
            blk["instructions"] = out
    return bir


def _split_sync_waits(bir: dict) -> dict:
    """Hoist extra per-instruction semaphore waits into standalone
    EventSemaphore instructions.

    The walrus build in this container encodes at most ONE sync wait per
    instruction ("Too many sync wait commands" otherwise); Tile attaches
    one wait per dependency.  An engine-level standalone wait immediately
    before the instruction is semantically identical (the engine stalls
    either way), so keep the last wait inline and hoist the rest.
    """
    n = 0
    for fn in bir.get("functions", []):
        for blk in fn.get("blocks", []):
            out = []
            for inst in blk.get("instructions", []):
                si = inst.get("sync_info") or {}
                ow = si.get("on_wait") or []
                if len(ow) > 1:
                    for w in ow[:-1]:
                        n += 1
                        out.append({
                            "debug": inst.get("debug", 0),
                            "engine": inst["engine"],
                            "ins": [],
                            "outs": [],
                            "name": f"hoistw_{n}_{inst['name']}",
                            "opcode": "EventSemaphore",
                            "sync_info": {"on_update": [], "on_wait": [w]},
                        })
                    si["on_wait"] = [ow[-1]]
                out.append(inst)
            blk["instructions"] = out
    return bir


def _patch_json(nc: bass.Bass) -> None:
    orig = nc.to_json_bytes

    def patched() -> bytes:
        return _json.dumps(
            _split_sync_waits(
                _elide_duplicate_ldweights(
                    _elide_redundant_waits(_json.loads(orig()))))
        ).encode()

    nc.to_json_bytes = patched


def _build_nc(bpc: int) -> bass.Bass:
    """Bass program for one core: GRU over a [H, bpc] feature-major shard.

    Per-input-chunk byte layout (uint8 dram tensor `inp`, 4*csz bytes):
        [x_f8 (csz) | h_f8 (csz) | h_bf16 (2*csz)]
    x_f8|h_f8 doubles as the DoubleRow rhs [K, 2, N] (k-tile stride csz).

    Gate matmuls: r/z via fp8 DoubleRow (merges the W_ih@x and W_hh@h
    contractions into one pass at 2 elem/cycle); i_n via plain fp8;
    h_n in bf16 (its path feeds tanh directly, keep it accurate).

    The DVE STT writes t = (h_n + b_hn)*r back INTO the p_hn PSUM bank
    (f32, in-place); the i_n matmuls then accumulate onto it with
    start=False one period later, so `pre = t + i_n` happens on the PE
    for free and tanh reads the finished pre-activation from PSUM.

    Software-pipelined per-engine stream order (period k):
        PE : inMM(k-1), rMM(k), zMM(k), hnMM(k)
        ACT: tanh(k-1), sig_r(k), sig_z(k)
        DVE: m(k-2), out(k-2), d(k-1), STT(k)
    so every instruction's dependencies were issued >= 1 period ago.
    """
    assert bpc % 256 == 0
    f32 = mybir.dt.float32
    bf16 = mybir.dt.bfloat16
    f8e4 = mybir.dt.float8e4
    u8 = mybir.dt.uint8
    sig = mybir.ActivationFunctionType.Sigmoid
    tanh = mybir.ActivationFunctionType.Tanh
    add_op = mybir.AluOpType.add
    mult_op = mybir.AluOpType.mult
    DR = mybir.MatmulPerfMode.DoubleRow

    nc = bass.Bass()
    inp = nc.declare_dram_parameter("inp", [H, 4 * bpc], u8, isOutput=False)
    # packed [w_ih_r | w_hh_r | biases | w_ih_z | w_hh_z | w_ih_n | w_hh_n]
    # (bf16 bytes) — everything the FIRST matmul + sigmoid need is a
    # contiguous 66KB prefix.
    # bias columns: 0 = b_ih_r + b_hh_r, 1 = b_ih_z + b_hh_z, 2 = b_hh_n, 3 = b_ih_n
    wb = nc.declare_dram_parameter("wb", [H, 12 * H + 8], u8, isOutput=False)
    outT = nc.declare_dram_parameter("outT", [H, bpc], bf16, isOutput=True)

    with ExitStack() as ctx:
        tc = ctx.enter_context(tile.TileContext(nc))
        singles = ctx.enter_context(tc.tile_pool(name="singles", bufs=1))
        io = ctx.enter_context(tc.tile_pool(name="io", bufs=1))
        mids = ctx.enter_context(tc.tile_pool(name="mids", bufs=6))
        outs = ctx.enter_context(tc.tile_pool(name="outs", bufs=4))
        psum = ctx.enter_context(tc.tile_pool(name="psum", bufs=1, space="PSUM"))

        # weights + biases land first via sync-HWDGE; the first descriptor
        # carries only what the first matmul + sigmoid need (r-gate
        # weights + biases, 66KB) so compute starts ~1us earlier, the
        # second carries the z/n weights (needed one period later)
        wb_sb = singles.tile([H, 12 * H + 8], u8)
        cut = 4 * H + 8              # bytes: w_ih_r + w_hh_r + biases
        nc.sync.dma_start(out=wb_sb[:, :cut], in_=wb[:, :cut])
        nc.sync.dma_start(out=wb_sb[:, cut:], in_=wb[:, cut:])
        wb_bf = wb_sb[:, :].bitcast(bf16)                            # [H, 6H+4]
        w_ihr = wb_bf[:, 0:H]
        w_hhr = wb_bf[:, H : 2 * H]
        b_sb = wb_bf[:, 2 * H : 2 * H + 4]
        w_ihz = wb_bf[:, 2 * H + 4 : 3 * H + 4]
        w_hhz = wb_bf[:, 3 * H + 4 : 4 * H + 4]
        w_ihn = wb_bf[:, 4 * H + 4 : 5 * H + 4]
        w_hhn = wb_bf[:, 5 * H + 4 : 6 * H + 4]

        # dummy sigmoid fires the ~2.7us ACT table load immediately, so it
        # overlaps the DMA ramp instead of stalling the first real sigmoid
        # (memset first so it doesn't wait on any DMA)
        warm_sb = singles.tile([H, 1], f32)
        nc.vector.memset(warm_sb, 0.0)
        nc.scalar.activation(out=warm_sb, in_=warm_sb,
                             func=sig, bias=0.0, scale=1.0)

        # Pre-issue EVERY input DMA before any compute/output instruction
        # lands in the sync queue: descriptor generation costs ~0.7us on
        # the issuing engine, and an out-DMA interleaved in the stream
        # would stall all later in-DMA issues behind that chunk's compute.
        # All chunks get distinct tiles (whole input fits in SBUF).
        parts = []
        for ci, (c0, csz) in enumerate(_dma_chunks(bpc)):
            t = io.tile([H, 4 * csz], u8, tag=f"c{ci}")
            nc.sync.dma_start(out=t, in_=inp[:, 4 * c0 : 4 * c0 + 4 * csz])
            parts.append((c0, csz, t))

        def views(lo, n):
            """(x_bf [H,n], h_bf [H,n]) for cols [lo, lo+n)."""
            for c0, csz, t in parts:
                if c0 <= lo and lo + n <= c0 + csz:
                    r0 = lo - c0
                    return (t[:, 0 : 2 * csz].bitcast(bf16)[:, r0 : r0 + n],
                            t[:, 2 * csz : 4 * csz].bitcast(bf16)[:, r0 : r0 + n])
            raise AssertionError((lo, n))

        tiles = _tiles(bpc)
        n_tiles = len(tiles)
        state = {}

        # pair consecutive full tiles from the same DMA chunk: the z/n
        # buffers and the whole blend (d, m, out, output DMA) then run at
        # [H, 2048] granularity — one instruction instead of two saves
        # the per-op fixed cost + semaphores on the pacing engines.
        def chunk_of(pos):
            for c0, csz, _ in parts:
                if c0 <= pos < c0 + csz:
                    return c0
            raise AssertionError(pos)

        pair_even, pair_odd = set(), set()
        i = 0
        while i + 1 < n_tiles:
            if (tiles[i][1] == NTILE and tiles[i + 1][1] == NTILE
                    and chunk_of(tiles[i][0]) == chunk_of(tiles[i + 1][0])):
                pair_even.add(i)
                pair_odd.add(i + 1)
                i += 2
            else:
                i += 1
        # measured: pair-wide blend (2048-wide d/m/out) bursts the DVE
        # and costs ~1.5us; keep only the output-DMA pairing.  The last
        # pair stays unpaired so the final transfers are small.
        dma_pair_even, dma_pair_odd = pair_even, pair_odd
        if dma_pair_even:
            le = max(dma_pair_even)
            dma_pair_even = dma_pair_even - {le}
            dma_pair_odd = dma_pair_odd - {le + 1}
        pair_even, pair_odd = set(), set()
        o_ctx = {}

        def stage_front(ti):
            t0, tsz = tiles[ti]
            x_sb, h_sb = views(t0, tsz)

            p_r = psum.tile([H, tsz], f32, tag="p_r")
            p_z = psum.tile([H, tsz], f32, tag="p_z")
            # double-buffered: tile k+1's h_n matmuls need not wait for
            # tile k's tanh to drain the bank (2+2+2x2 = all 8 banks)
            p_hn = psum.tile([H, tsz], f32, tag="p_hn", bufs=2)

            # weight-major order: back-to-back matmuls share stationary
            # weights, so the BIR pass below drops the duplicate
            # LDWEIGHTS (~100ns of PE issue each)
            for wih_g, whh_g, dst in ((w_ihr, w_hhr, p_r),
                                      (w_ihz, w_hhz, p_z)):
                for src_sb, w_sb, st in (
                    (x_sb, wih_g, True),
                    (h_sb, whh_g, False),
                ):
                    for q0 in range(0, tsz, 512):
                        qs = slice(q0, q0 + min(512, tsz - q0))
                        nc.tensor.matmul(dst[:, qs], w_sb, src_sb[:, qs],
                                         start=st, stop=not st,
                                         skip_group_check=True)
            for q0 in range(0, tsz, 512):
                qs = slice(q0, q0 + min(512, tsz - q0))
                nc.tensor.matmul(p_hn[:, qs], w_hhn,
                                 h_sb[:, qs], start=True, stop=False,
                                 skip_group_check=True)

            r_t = mids.tile([H, tsz], bf16, tag="r")
            if ti in pair_even:
                z_pair = mids.tile([H, 2 * NTILE], bf16, tag="z")
                state[ti] = st = {"z_pair": z_pair}
                z_t = z_pair[:, 0:NTILE]
            elif ti in pair_odd:
                st = state[ti] = {}
                z_t = state[ti - 1]["z_pair"][:, NTILE : 2 * NTILE]
            else:
                st = state[ti] = {}
                z_t = mids.tile([H, tsz], bf16, tag="z")
            nc.scalar.activation(out=r_t, in_=p_r, func=sig,
                                 bias=b_sb[:, 0:1], scale=1.0)
            nc.scalar.activation(out=z_t, in_=p_z, func=sig,
                                 bias=b_sb[:, 1:2], scale=1.0)

            # in-place: p_hn <- (p_hn + b_hn) * r   (f32, stays in PSUM)
            nc.vector.scalar_tensor_tensor(
                out=p_hn, in0=p_hn, scalar=b_sb[:, 2:3], in1=r_t,
                op0=add_op, op1=mult_op)
            st.update({"h": h_sb, "x": x_sb, "z": z_t, "p_hn": p_hn})

        def stage_in_mm(ti):
            """i_n matmuls accumulate onto the STT result (one period
            after front; first in the PE stream so tanh unblocks early)."""
            t0, tsz = tiles[ti]
            st = state[ti]
            p_hn, x_sb = st["p_hn"], st["x"]
            for q0 in range(0, tsz, 512):
                qs = slice(q0, q0 + min(512, tsz - q0))
                nc.tensor.matmul(p_hn[:, qs], w_ihn,
                                 x_sb[:, qs], start=False, stop=True,
                                 skip_group_check=True)

        def stage_tanh(ti):
            t0, tsz = tiles[ti]
            st = state[ti]
            if ti in pair_even:
                n_pair = mids.tile([H, 2 * NTILE], bf16, tag="n")
                st["n_pair"] = n_pair
                n_t = n_pair[:, 0:NTILE]
            elif ti in pair_odd:
                n_t = state[ti - 1]["n_pair"][:, NTILE : 2 * NTILE]
            else:
                n_t = mids.tile([H, tsz], bf16, tag="n")
            nc.scalar.activation(out=n_t, in_=st["p_hn"], func=tanh,
                                 bias=b_sb[:, 3:4], scale=1.0)
            st["n"] = n_t

        def stage_d(ti):
            if ti in pair_even or ti in pair_odd:
                return               # handled pair-wide in stage_blend
            t0, tsz = tiles[ti]
            st = state[ti]
            d_t = mids.tile([H, tsz], bf16, tag="d")
            # measured: GpSimd here costs +7us (power-throttle window
            # shrinks); keep the whole blend on DVE
            nc.vector.tensor_sub(out=d_t, in0=st["h"], in1=st["n"])
            st["d"] = d_t

        def stage_blend(ti):
            """d = h-n, m = z*d, out = n + m, DMA (two periods after
            front).  For paired tiles everything runs once per pair at
            [H, 2048]."""
            t0, tsz = tiles[ti]
            if ti in pair_even:
                return               # odd member does the work
            if ti in pair_odd:
                ste = state.pop(ti - 1)
                state.pop(ti)
                z_pair = ste["z_pair"]
                n_pair = ste["n_pair"]
                e0 = tiles[ti - 1][0]
                h_pair = views(e0, 2 * NTILE)[1]
                d_t = mids.tile([H, 2 * NTILE], bf16, tag="d")
                m_t = mids.tile([H, 2 * NTILE], bf16, tag="m")
                o_t = outs.tile([H, 2 * NTILE], bf16, tag="o")
                nc.vector.tensor_sub(out=d_t, in0=h_pair, in1=n_pair)
                nc.vector.tensor_mul(out=m_t, in0=z_pair, in1=d_t)
                nc.vector.tensor_add(out=o_t, in0=n_pair, in1=m_t)
                nc.sync.dma_start(out=outT[:, e0 : e0 + 2 * NTILE], in_=o_t)
                return
            st = state.pop(ti)
            m_t = mids.tile([H, tsz], bf16, tag="m")
            nc.vector.tensor_mul(out=m_t, in0=st["z"], in1=st["d"])
            if ti in dma_pair_even:
                o_pair = outs.tile([H, 2 * NTILE], bf16, tag="o")
                o_ctx[ti + 1] = o_pair
                nc.vector.tensor_add(out=o_pair[:, 0:NTILE], in0=st["n"],
                                     in1=m_t)
            elif ti in dma_pair_odd:
                o_pair = o_ctx.pop(ti)
                nc.vector.tensor_add(out=o_pair[:, NTILE : 2 * NTILE],
                                     in0=st["n"], in1=m_t)
                nc.sync.dma_start(
                    out=outT[:, t0 - NTILE : t0 + tsz], in_=o_pair)
            else:
                o_t = outs.tile([H, tsz], bf16, tag="o")
                nc.vector.tensor_add(out=o_t, in0=st["n"], in1=m_t)
                nc.sync.dma_start(out=outT[:, t0 : t0 + tsz], in_=o_t)

        for ti in range(n_tiles):
            if ti >= 1:
                stage_in_mm(ti - 1)
                stage_tanh(ti - 1)
            if ti >= 2:
                stage_blend(ti - 2)
            if ti >= 1:
                stage_d(ti - 1)
            stage_front(ti)
        stage_in_mm(n_tiles - 1)
        stage_tanh(n_tiles - 1)
        stage_blend(n_tiles - 2)
        stage_d(n_tiles - 1)
        stage_blend(n_tiles - 1)

    _patch_json(nc)
    return nc


def _get_nc(bpc: int) -> bass.Bass:
    if bpc not in _NC_CACHE:
        _NC_CACHE[bpc] = _build_nc(bpc)
    return _NC_CACHE[bpc]


def kernel(node_ids, messages, memory, W_ih, W_hh, b_ih, b_hh):
    global LAST_RESULT
    node_ids = np.asarray(node_ids)
    messages = np.asarray(messages, dtype=np.float32)
    memory = np.asarray(memory, dtype=np.float32)
    W_ih = np.asarray(W_ih, dtype=np.float32)
    W_hh = np.asarray(W_hh, dtype=np.float32)
    b_ih = np.asarray(b_ih, dtype=np.float32)
    b_hh = np.asarray(b_hh, dtype=np.float32)

    B = node_ids.shape[0]
    per = -(-B // N_CORES)                       # rows per core (unpadded)
    bpc = -(-per // 256) * 256                   # padded to 256 multiple
    nc = _get_nc(bpc)
    chunks = _dma_chunks(bpc)

    current = memory[node_ids]                   # [B, H] host gather

    # weights: [w_ih_r | w_hh_r | biases | w_ih_z | w_hh_z | w_ih_n | w_hh_n]
    w_ihT = W_ih.T
    w_hhT = W_hh.T
    wbf = np.empty((H, 6 * H + 4), dtype=np.float32)
    wbf[:, 0:H] = w_ihT[:, 0:H]
    wbf[:, H : 2 * H] = w_hhT[:, 0:H]
    wbf[:, 2 * H + 0] = b_ih[0:H] + b_hh[0:H]
    wbf[:, 2 * H + 1] = b_ih[H : 2 * H] + b_hh[H : 2 * H]
    wbf[:, 2 * H + 2] = b_hh[2 * H : 3 * H]
    wbf[:, 2 * H + 3] = b_ih[2 * H : 3 * H]
    wbf[:, 2 * H + 4 : 3 * H + 4] = w_ihT[:, H : 2 * H]
    wbf[:, 3 * H + 4 : 4 * H + 4] = w_hhT[:, H : 2 * H]
    wbf[:, 4 * H + 4 : 5 * H + 4] = w_ihT[:, 2 * H : 3 * H]
    wbf[:, 5 * H + 4 : 6 * H + 4] = w_hhT[:, 2 * H : 3 * H]
    wb = wbf.astype(BF16).view(np.uint8)

    in_maps = []
    for c in range(N_CORES):
        lo = c * per
        hi = min(lo + per, B)
        xT = np.zeros((H, bpc), dtype=np.float32)
        hT = np.zeros((H, bpc), dtype=np.float32)
        if hi > lo:
            xT[:, : hi - lo] = messages[lo:hi].T
            hT[:, : hi - lo] = current[lo:hi].T
        x_bf = xT.astype(BF16)
        h_bf = hT.astype(BF16)
        inp = np.empty((H, 4 * bpc), dtype=np.uint8)
        for c0, csz in chunks:
            o = 4 * c0
            inp[:, o : o + 2 * csz] = np.ascontiguousarray(
                x_bf[:, c0 : c0 + csz]).view(np.uint8)
            inp[:, o + 2 * csz : o + 4 * csz] = np.ascontiguousarray(
                h_bf[:, c0 : c0 + csz]).view(np.uint8)
        in_maps.append({"inp": inp, "wb": wb})

    res = run_bass_kernel_spmd(nc, in_maps, list(range(N_CORES)))
    LAST_RESULT = res

    updated = np.empty((B, H), dtype=np.float32)
    for c in range(N_CORES):
        lo = c * per
        hi = min(lo + per, B)
        if hi > lo:
            updated[lo:hi] = res.results[c]["outT"][:, : hi - lo].T.astype(np.float32)

    new_memory = memory.copy()
    new_memory[node_ids] = updated
    return new_memory
